# revision 19
# baseline (speedup 1.0000x reference)
"""MoE (8 experts, top-2, SwiGLU FFN) Trainium2 kernel — expert-parallel.

Sharding (per the expert-parallel hint): the host computes the router
(float64 numpy — bit-stable top-2 vs the fp32 reference; the smallest
2nd-vs-3rd softmax gap ~2e-5 is far above fp32 matmul noise) and
dispatches: core e receives ONLY the tokens routed to expert e (gathered,
padded to capacity C = roundup(max_e count_e, 128)) plus that expert's
weights. Each core runs the SwiGLU FFN for its expert and scales rows by
the combine weight; the host scatter-adds the two expert contributions
per token and adds the cw1*b2[i1]+cw2*b2[i2] bias term.

vs the dense data-parallel baseline this is ~4x less matmul work
(top-2-of-8 sparsity, minus ~12% capacity padding) and ~5x less weight
DMA per core (each core loads 1 expert's weights, not all 8).

Schedule notes (cost-model driven; see TimelineSim):
 - Single fused mm1 pass: per token-chunk cc, dt-outer over 8 PSUM banks
   (4 for h=w1@x, 4 for u=w3@x) so the PE consumes the interleaved
   (w1[dt], w3[dt], xg[cc0,dt]) DMA stream as it lands.
 - gu = silu(h+b1) * (u+b3): one ACT op + one DVE op per (cc, ht); no
   intermediate g tensor kept across phases.
 - Every dma_start costs its queue's sequencer ~0.6us and all transfers
   serialize on the shared DMA engines (~360 B/ns), so: loads are per-dt
   (streaming granularity) on the sync queue, stores are 9 big [128, D]
   tiles alternating the otherwise-idle act/gpsimd queues, small tensors
   (b1, b3, cw) ride gpsimd.
 - Two discarded fp32 matmuls on a memset tile warm the PE p-state ramp
   while the first DMAs land.

Per-core layouts (partition dim first):
  xg  [128(d%128), 8(d//128), C]      moving operand of mm1 (f32r)
  w1/w3 [128(d%128), 8, 512(h)]       stationary tiles [d,h] for mm1
  gu  [128(h%128), 4(h//128), C]      silu(h+b1)*(u+b3), mm2 stationary
  w2  [128(h%128), 4, 1024(d)]        moving operand of mm2
  y   PSUM [128(t%128), 512(d-chunk)] accum over h-tiles; *cw on evict
  out dram [CT, 128, 1024]            host reshape to [C, 1024]
"""

import ml_dtypes
import numpy as np

import concourse.bass as bass
import concourse.bacc as bacc
import concourse.mybir as mybir
import concourse.tile as tile

D, H, E, T = 1024, 512, 8, 4096
NCORES = 8
DT = D // 128               # 8 d-tiles
HT = H // 128               # 4 h-tiles
DC = D // 512               # 2 d-chunks for mm2 psum
N_WARM = 1
F32 = mybir.dt.float32
F32R = mybir.dt.float32r
BF16 = mybir.dt.bfloat16
AX = mybir.AluOpType
AF = mybir.ActivationFunctionType


def _chunks(C):
    """Split C into multiples of 128, each in [256, 512] (C >= 256)."""
    n = max(1, (C + 511) // 512)
    base = C // n // 128 * 128
    sizes = [base] * n
    rem = C - base * n
    i = 0
    while rem > 0:
        sizes[i] += 128
        rem -= 128
        i = (i + 1) % n
    out, c0 = [], 0
    for s in sizes:
        out.append((c0, s))
        c0 += s
    return out


def build_nc(C):
    CT = C // 128
    ccs = _chunks(C)
    nc = bacc.Bacc("TRN2", target_bir_lowering=False, debug=False,
                   num_devices=NCORES)

    # blob packs [w1 | w3 | xg] along the free dim so each dt streams in as
    # ONE dma_start (queue dispatch costs ~0.6us per DMA; transfers ~1us)
    W = 2 * H + C
    blob = nc.dram_tensor("blob", [DT, 128, W], BF16, kind="ExternalInput")
    w2t = nc.dram_tensor("w2t", [HT, 128, D], BF16, kind="ExternalInput")
    b1t = nc.dram_tensor("b1t", [HT, 128], F32, kind="ExternalInput")
    b3t = nc.dram_tensor("b3t", [HT, 128], F32, kind="ExternalInput")
    cwt = nc.dram_tensor("cwt", [CT, 128], F32, kind="ExternalInput")
    out = nc.dram_tensor("out", [CT, 128, D], F32, kind="ExternalOutput")

    with tile.TileContext(nc) as tc:
        with (
            tc.tile_pool(name="singles", bufs=1) as singles,
            tc.tile_pool(name="gpool", bufs=4) as gpool,
            tc.tile_pool(name="opool", bufs=10) as opool,
            tc.tile_pool(name="pmm", bufs=8, space="PSUM") as pmm,
        ):
            # ---- DMA issue order (sync queue = big load streams) ------------
            # wx_sb[:, dt, 0:H]=w1, [H:2H]=w3, [2H:2H+C]=xg
            wx_sb = singles.tile([128, DT, W], BF16)
            cs0 = ccs[0][1]
            for dt in range(DT):
                nc.sync.dma_start(out=wx_sb[:, dt, 0:2 * H + cs0],
                                  in_=blob.ap()[dt][:, 0:2 * H + cs0])
            blob_r = blob.ap().rearrange("a p t -> p a t")
            for (c0, cs) in ccs[1:]:
                nc.sync.dma_start(out=wx_sb[:, :, 2 * H + c0:2 * H + c0 + cs],
                                  in_=blob_r[:, :, 2 * H + c0:2 * H + c0 + cs])
            w2_sb = singles.tile([128, HT, D], BF16)
            nc.sync.dma_start(out=w2_sb, in_=w2t.ap().rearrange("a p d -> p a d"))

            # small tensors on the gpsimd queue (no head-of-line blocking)
            b1_sb = singles.tile([128, HT], F32)
            nc.gpsimd.dma_start(out=b1_sb, in_=b1t.ap().rearrange("h p -> p h"))
            b3_sb = singles.tile([128, HT], F32)
            nc.gpsimd.dma_start(out=b3_sb, in_=b3t.ap().rearrange("h p -> p h"))
            cw_sb = singles.tile([128, CT], F32)
            nc.gpsimd.dma_start(out=cw_sb, in_=cwt.ap().rearrange("t p -> p t"))

            # ---- PE warm-up: fp32 matmuls on a memset tile (no DMA dep) -----
            junkf = singles.tile([128, 512], F32)
            nc.vector.memset(junkf, 1.0)
            p_warm = pmm.tile([128, 512], F32, tag="mm")
            for _ in range(N_WARM):
                nc.tensor.matmul(p_warm, junkf[:, 0:128], junkf,
                                 start=True, stop=True)

            gu_sb = singles.tile([128, HT, C], BF16)

            # ---- fused mm1: h = w1@x, u = w3@x ; gu = silu(h+b1)*(u+b3) -----
            for (c0, cs) in ccs:
                cc = slice(c0, c0 + cs)
                phs = [pmm.tile([128, cs], F32, tag="mm", name=f"ph{c0}_{h}")
                       for h in range(HT)]
                pus = [pmm.tile([128, cs], F32, tag="mm", name=f"pu{c0}_{h}")
                       for h in range(HT)]
                xc = slice(2 * H + c0, 2 * H + c0 + cs)
                for dt in range(DT):
                    for ht in range(HT):
                        h1 = slice(ht * 128, (ht + 1) * 128)
                        h3 = slice(H + ht * 128, H + (ht + 1) * 128)
                        nc.tensor.matmul(phs[ht], wx_sb[:, dt, h1],
                                         wx_sb[:, dt, xc],
                                         start=(dt == 0), stop=(dt == DT - 1))
                        nc.tensor.matmul(pus[ht], wx_sb[:, dt, h3],
                                         wx_sb[:, dt, xc],
                                         start=(dt == 0), stop=(dt == DT - 1))
                for ht in range(HT):
                    g_tmp = gpool.tile([128, cs], F32, tag="g",
                                       name=f"g{c0}_{ht}")
                    nc.scalar.activation(g_tmp, phs[ht], AF.Silu,
                                         bias=b1_sb[:, ht:ht + 1], scale=1.0)
                    nc.vector.scalar_tensor_tensor(gu_sb[:, ht, cc], pus[ht],
                                                   b3_sb[:, ht:ht + 1], g_tmp,
                                                   op0=AX.add, op1=AX.mult)

            # ---- mm2: y = gu.T @ w2 ; out = cw * y --------------------------
            # per-(tt,dc) stores drain during mm2 on the idle act/pool queues
            for tt in range(CT):
                ts_ = slice(tt * 128, (tt + 1) * 128)
                for dc in range(DC):
                    ds_ = slice(dc * 512, (dc + 1) * 512)
                    p_y = pmm.tile([128, 512], F32, tag="mm")
                    for ht in range(HT):
                        nc.tensor.matmul(p_y, gu_sb[:, ht, ts_],
                                         w2_sb[:, ht, ds_],
                                         start=(ht == 0), stop=(ht == HT - 1))
                    o_sb = opool.tile([128, 512], F32, tag="o")
                    if dc % 2 == 0:
                        nc.vector.tensor_scalar_mul(o_sb, p_y,
                                                    cw_sb[:, tt:tt + 1])
                    else:
                        nc.scalar.activation(o_sb, p_y, AF.Copy,
                                             scale=cw_sb[:, tt:tt + 1])
                    q = nc.scalar if (tt * DC + dc) % 2 == 0 else nc.sync
                    q.dma_start(out=out.ap()[tt][:, ds_], in_=o_sb)

    nc.compile()
    return nc


_NC_CACHE = {}


def _get_nc(C):
    if C not in _NC_CACHE:
        _NC_CACHE[C] = build_nc(C)
    return _NC_CACHE[C]


def route(x, router_w):
    """Host router in float64: top-2 selection is bit-stable vs the fp32
    reference (min 2nd-vs-3rd softmax gap ~2e-5 >> fp32 matmul noise)."""
    xt = np.asarray(x, np.float64).reshape(T, D)
    logits = xt @ np.asarray(router_w, np.float64).T          # [T, E]
    logits -= logits.max(axis=1, keepdims=True)
    ex = np.exp(logits)
    sm = ex / ex.sum(axis=1, keepdims=True)                   # [T, E]
    order = np.argsort(-sm, axis=1, kind="stable")
    i1, i2 = order[:, 0], order[:, 1]
    ar = np.arange(T)
    cw1 = sm[ar, i1].astype(np.float32)
    cw2 = sm[ar, i2].astype(np.float32)
    return i1, i2, cw1, cw2


def prepare(x, router_w, w1, b1, w3, b3, w2, b2):
    """Host routing + per-core input packing. Returns (C, in_maps, ...)."""
    xt = np.asarray(x, np.float32).reshape(T, D)
    i1, i2, cw1, cw2 = route(x, router_w)
    toks, cws = [], []
    for e in range(E):
        m1, m2 = i1 == e, i2 == e
        tok = np.nonzero(m1 | m2)[0]
        cw = np.where(m1, cw1, cw2)[tok]
        toks.append(tok)
        cws.append(cw.astype(np.float32))
    counts = [len(t) for t in toks]
    C = max(256, int(np.ceil(max(counts) / 128) * 128))
    CT = C // 128

    in_maps = []
    for e in range(E):
        n = counts[e]
        xgf = np.zeros((C, D), np.float32)
        xgf[:n] = xt[toks[e]]
        cwf = np.zeros(C, np.float32)
        cwf[:n] = cws[e]
        w1T = np.asarray(w1[e], np.float32).T.reshape(DT, 128, H)
        w3T = np.asarray(w3[e], np.float32).T.reshape(DT, 128, H)
        xgT = xgf.T.reshape(DT, 128, C)
        blob = np.concatenate([w1T, w3T, xgT], axis=2)
        in_maps.append({
            "blob": np.ascontiguousarray(blob.astype(ml_dtypes.bfloat16)),
            "w2t": np.ascontiguousarray(
                np.asarray(w2[e], np.float32).T).astype(
                    ml_dtypes.bfloat16).reshape(HT, 128, D),
            "b1t": np.asarray(b1[e], np.float32).reshape(HT, 128),
            "b3t": np.asarray(b3[e], np.float32).reshape(HT, 128),
            "cwt": cwf.reshape(CT, 128),
        })
    return C, in_maps, toks, (i1, i2, cw1, cw2)


def combine(results, toks, route_info, b2, C):
    """Scatter-add per-expert outputs (already cw-scaled) + per-expert
    bias term cw1*b2[i1] + cw2*b2[i2]."""
    i1, i2, cw1, cw2 = route_info
    acc = np.zeros((T, D), np.float32)
    for e in range(E):
        y = results[e]["out"].reshape(C, D)
        acc[toks[e]] += y[:len(toks[e])]
    b2f = np.asarray(b2, np.float32)
    acc += cw1[:, None] * b2f[i1] + cw2[:, None] * b2f[i2]
    return acc.reshape(4, 1024, D)


def kernel(x, router_w, w1, b1, w3, b3, w2, b2):
    from concourse.bass_utils import run_bass_kernel_spmd

    C, in_maps, toks, route_info = prepare(x, router_w, w1, b1, w3, b3, w2, b2)
    nc = _get_nc(C)
    res = run_bass_kernel_spmd(nc, in_maps, core_ids=list(range(NCORES)))
    return combine(res.results, toks, route_info, b2, C)


# revision 21
# speedup vs baseline: 4.6991x; 4.6991x over previous
"""MoE (8 experts, top-2, SwiGLU FFN) Trainium2 kernel — expert-parallel.

Sharding (per the expert-parallel hint): the host computes the router
(float64 numpy — bit-stable top-2 vs the fp32 reference; the smallest
2nd-vs-3rd softmax gap ~2e-5 is far above fp32 matmul noise) and
dispatches: core e receives ONLY the tokens routed to expert e (gathered,
padded to capacity C = roundup(max_e count_e, 128)) plus that expert's
weights. Each core runs the SwiGLU FFN for its expert and scales rows by
the combine weight; the host scatter-adds the two expert contributions
per token and adds the cw1*b2[i1]+cw2*b2[i2] bias term.

vs the dense data-parallel baseline this is ~4x less matmul work
(top-2-of-8 sparsity, minus ~12% capacity padding) and ~5x less weight
DMA per core (each core loads 1 expert's weights, not all 8).

Schedule notes (cost-model driven; see TimelineSim):
 - Single fused mm1 pass: per token-chunk cc, dt-outer over 8 PSUM banks
   (4 for h=w1@x, 4 for u=w3@x) so the PE consumes the interleaved
   (w1[dt], w3[dt], xg[cc0,dt]) DMA stream as it lands.
 - gu = silu(h+b1) * (u+b3): one ACT op + one DVE op per (cc, ht); no
   intermediate g tensor kept across phases.
 - Every dma_start costs its queue's sequencer ~0.6us and all transfers
   serialize on the shared DMA engines (~360 B/ns), so: loads are per-dt
   (streaming granularity) on the sync queue, stores are 9 big [128, D]
   tiles alternating the otherwise-idle act/gpsimd queues, small tensors
   (b1, b3, cw) ride gpsimd.
 - Two discarded fp32 matmuls on a memset tile warm the PE p-state ramp
   while the first DMAs land.

Per-core layouts (partition dim first):
  xg  [128(d%128), 8(d//128), C]      moving operand of mm1 (f32r)
  w1/w3 [128(d%128), 8, 512(h)]       stationary tiles [d,h] for mm1
  gu  [128(h%128), 4(h//128), C]      silu(h+b1)*(u+b3), mm2 stationary
  w2  [128(h%128), 4, 1024(d)]        moving operand of mm2
  y   PSUM [128(t%128), 512(d-chunk)] accum over h-tiles; *cw on evict
  out dram [CT, 128, 1024] bf16       host reshape to [C, 1024]
"""

import ml_dtypes
import numpy as np

import concourse.bass as bass
import concourse.bacc as bacc
import concourse.mybir as mybir
import concourse.tile as tile

D, H, E, T = 1024, 512, 8, 4096
NCORES = 8
DT = D // 128               # 8 d-tiles
HT = H // 128               # 4 h-tiles
DC = D // 512               # 2 d-chunks for mm2 psum
N_WARM = 1
F32 = mybir.dt.float32
F32R = mybir.dt.float32r
BF16 = mybir.dt.bfloat16
AX = mybir.AluOpType
AF = mybir.ActivationFunctionType


def _chunks(C):
    """Split C into multiples of 128, each in [256, 512] (C >= 256)."""
    n = max(1, (C + 511) // 512)
    base = C // n // 128 * 128
    sizes = [base] * n
    rem = C - base * n
    i = 0
    while rem > 0:
        sizes[i] += 128
        rem -= 128
        i = (i + 1) % n
    out, c0 = [], 0
    for s in sizes:
        out.append((c0, s))
        c0 += s
    return out


def build_nc(C):
    CT = C // 128
    ccs = _chunks(C)
    nc = bacc.Bacc("TRN2", target_bir_lowering=False, debug=False,
                   num_devices=NCORES)

    # blob packs [w1 | w3 | xg] along the free dim so each dt streams in as
    # ONE dma_start (queue dispatch costs ~0.6us per DMA; transfers ~1us)
    W = 2 * H + C
    blob = nc.dram_tensor("blob", [DT, 128, W], BF16, kind="ExternalInput")
    w2t = nc.dram_tensor("w2t", [HT, 128, D], BF16, kind="ExternalInput")
    b1t = nc.dram_tensor("b1t", [HT, 128], F32, kind="ExternalInput")
    b3t = nc.dram_tensor("b3t", [HT, 128], F32, kind="ExternalInput")
    cwt = nc.dram_tensor("cwt", [CT, 128], F32, kind="ExternalInput")
    out = nc.dram_tensor("out", [CT, 128, D], BF16, kind="ExternalOutput")

    with tile.TileContext(nc) as tc:
        with (
            tc.tile_pool(name="singles", bufs=1) as singles,
            tc.tile_pool(name="gpool", bufs=4) as gpool,
            tc.tile_pool(name="opool", bufs=10) as opool,
            tc.tile_pool(name="pmm", bufs=8, space="PSUM") as pmm,
        ):
            # ---- DMA issue order (sync queue = big load streams) ------------
            # wx_sb[:, dt, 0:H]=w1, [H:2H]=w3, [2H:2H+C]=xg
            wx_sb = singles.tile([128, DT, W], BF16)
            cs0 = ccs[0][1]
            for dt in range(DT):
                nc.sync.dma_start(out=wx_sb[:, dt, 0:2 * H + cs0],
                                  in_=blob.ap()[dt][:, 0:2 * H + cs0])
            blob_r = blob.ap().rearrange("a p t -> p a t")
            for (c0, cs) in ccs[1:]:
                nc.sync.dma_start(out=wx_sb[:, :, 2 * H + c0:2 * H + c0 + cs],
                                  in_=blob_r[:, :, 2 * H + c0:2 * H + c0 + cs])
            w2_sb = singles.tile([128, HT, D], BF16)
            nc.sync.dma_start(out=w2_sb, in_=w2t.ap().rearrange("a p d -> p a d"))

            # small tensors on the gpsimd queue (no head-of-line blocking)
            b1_sb = singles.tile([128, HT], F32)
            nc.gpsimd.dma_start(out=b1_sb, in_=b1t.ap().rearrange("h p -> p h"))
            b3_sb = singles.tile([128, HT], F32)
            nc.gpsimd.dma_start(out=b3_sb, in_=b3t.ap().rearrange("h p -> p h"))
            cw_sb = singles.tile([128, CT], F32)
            nc.gpsimd.dma_start(out=cw_sb, in_=cwt.ap().rearrange("t p -> p t"))

            # ---- PE warm-up: fp32 matmuls on a memset tile (no DMA dep) -----
            junkf = singles.tile([128, 512], F32)
            nc.vector.memset(junkf, 1.0)
            p_warm = pmm.tile([128, 512], F32, tag="mm")
            for _ in range(N_WARM):
                nc.tensor.matmul(p_warm, junkf[:, 0:128], junkf,
                                 start=True, stop=True)

            gu_sb = singles.tile([128, HT, C], BF16)

            # ---- fused mm1: h = w1@x, u = w3@x ; gu = silu(h+b1)*(u+b3) -----
            for (c0, cs) in ccs:
                cc = slice(c0, c0 + cs)
                phs = [pmm.tile([128, cs], F32, tag="mm", name=f"ph{c0}_{h}")
                       for h in range(HT)]
                pus = [pmm.tile([128, cs], F32, tag="mm", name=f"pu{c0}_{h}")
                       for h in range(HT)]
                xc = slice(2 * H + c0, 2 * H + c0 + cs)
                for dt in range(DT):
                    for ht in range(HT):
                        h1 = slice(ht * 128, (ht + 1) * 128)
                        h3 = slice(H + ht * 128, H + (ht + 1) * 128)
                        nc.tensor.matmul(phs[ht], wx_sb[:, dt, h1],
                                         wx_sb[:, dt, xc],
                                         start=(dt == 0), stop=(dt == DT - 1))
                        nc.tensor.matmul(pus[ht], wx_sb[:, dt, h3],
                                         wx_sb[:, dt, xc],
                                         start=(dt == 0), stop=(dt == DT - 1))
                for ht in range(HT):
                    g_tmp = gpool.tile([128, cs], F32, tag="g",
                                       name=f"g{c0}_{ht}")
                    nc.scalar.activation(g_tmp, phs[ht], AF.Silu,
                                         bias=b1_sb[:, ht:ht + 1], scale=1.0)
                    nc.vector.scalar_tensor_tensor(gu_sb[:, ht, cc], pus[ht],
                                                   b3_sb[:, ht:ht + 1], g_tmp,
                                                   op0=AX.add, op1=AX.mult)

            # ---- mm2: y = gu.T @ w2 ; out = cw * y --------------------------
            # per-(tt,dc) stores drain during mm2 on the idle act/pool queues
            for tt in range(CT):
                ts_ = slice(tt * 128, (tt + 1) * 128)
                for dc in range(DC):
                    ds_ = slice(dc * 512, (dc + 1) * 512)
                    p_y = pmm.tile([128, 512], F32, tag="mm")
                    for ht in range(HT):
                        nc.tensor.matmul(p_y, gu_sb[:, ht, ts_],
                                         w2_sb[:, ht, ds_],
                                         start=(ht == 0), stop=(ht == HT - 1))
                    o_sb = opool.tile([128, 512], BF16, tag="o")
                    if dc % 2 == 0:
                        nc.vector.tensor_scalar_mul(o_sb, p_y,
                                                    cw_sb[:, tt:tt + 1])
                    else:
                        nc.scalar.activation(o_sb, p_y, AF.Copy,
                                             scale=cw_sb[:, tt:tt + 1])
                    q = nc.scalar if (tt * DC + dc) % 2 == 0 else nc.sync
                    q.dma_start(out=out.ap()[tt][:, ds_], in_=o_sb)

    nc.compile()
    return nc


_NC_CACHE = {}


def _get_nc(C):
    if C not in _NC_CACHE:
        _NC_CACHE[C] = build_nc(C)
    return _NC_CACHE[C]


def route(x, router_w):
    """Host router in float64: top-2 selection is bit-stable vs the fp32
    reference (min 2nd-vs-3rd softmax gap ~2e-5 >> fp32 matmul noise)."""
    xt = np.asarray(x, np.float64).reshape(T, D)
    logits = xt @ np.asarray(router_w, np.float64).T          # [T, E]
    logits -= logits.max(axis=1, keepdims=True)
    ex = np.exp(logits)
    sm = ex / ex.sum(axis=1, keepdims=True)                   # [T, E]
    order = np.argsort(-sm, axis=1, kind="stable")
    i1, i2 = order[:, 0], order[:, 1]
    ar = np.arange(T)
    cw1 = sm[ar, i1].astype(np.float32)
    cw2 = sm[ar, i2].astype(np.float32)
    return i1, i2, cw1, cw2


def prepare(x, router_w, w1, b1, w3, b3, w2, b2):
    """Host routing + per-core input packing. Returns (C, in_maps, ...)."""
    xt = np.asarray(x, np.float32).reshape(T, D)
    i1, i2, cw1, cw2 = route(x, router_w)
    toks, cws = [], []
    for e in range(E):
        m1, m2 = i1 == e, i2 == e
        tok = np.nonzero(m1 | m2)[0]
        cw = np.where(m1, cw1, cw2)[tok]
        toks.append(tok)
        cws.append(cw.astype(np.float32))
    counts = [len(t) for t in toks]
    C = max(256, int(np.ceil(max(counts) / 128) * 128))
    CT = C // 128

    in_maps = []
    for e in range(E):
        n = counts[e]
        xgf = np.zeros((C, D), np.float32)
        xgf[:n] = xt[toks[e]]
        cwf = np.zeros(C, np.float32)
        cwf[:n] = cws[e]
        w1T = np.asarray(w1[e], np.float32).T.reshape(DT, 128, H)
        w3T = np.asarray(w3[e], np.float32).T.reshape(DT, 128, H)
        xgT = xgf.T.reshape(DT, 128, C)
        blob = np.concatenate([w1T, w3T, xgT], axis=2)
        in_maps.append({
            "blob": np.ascontiguousarray(blob.astype(ml_dtypes.bfloat16)),
            "w2t": np.ascontiguousarray(
                np.asarray(w2[e], np.float32).T).astype(
                    ml_dtypes.bfloat16).reshape(HT, 128, D),
            "b1t": np.asarray(b1[e], np.float32).reshape(HT, 128),
            "b3t": np.asarray(b3[e], np.float32).reshape(HT, 128),
            "cwt": cwf.reshape(CT, 128),
        })
    return C, in_maps, toks, (i1, i2, cw1, cw2)


def combine(results, toks, route_info, b2, C):
    """Scatter-add per-expert outputs (already cw-scaled) + per-expert
    bias term cw1*b2[i1] + cw2*b2[i2]."""
    i1, i2, cw1, cw2 = route_info
    acc = np.zeros((T, D), np.float32)
    for e in range(E):
        y = results[e]["out"].reshape(C, D).astype(np.float32)
        acc[toks[e]] += y[:len(toks[e])]
    b2f = np.asarray(b2, np.float32)
    acc += cw1[:, None] * b2f[i1] + cw2[:, None] * b2f[i2]
    return acc.reshape(4, 1024, D)


def kernel(x, router_w, w1, b1, w3, b3, w2, b2):
    from concourse.bass_utils import run_bass_kernel_spmd

    C, in_maps, toks, route_info = prepare(x, router_w, w1, b1, w3, b3, w2, b2)
    nc = _get_nc(C)
    res = run_bass_kernel_spmd(nc, in_maps, core_ids=list(range(NCORES)))
    return combine(res.results, toks, route_info, b2, C)
